# revision 4
# baseline (speedup 1.0000x reference)
"""Trainium2 Bass kernel for nn_MetaSDSA (spiking MetaFormer SDSA block).

Strategy
--------
* Data-parallel over batch: 8 cores x 2 samples each. Each core runs the full
  T=4 LIF recurrences for its samples, everything resident in SBUF.
* Channel-major layout: C=384 = 3 chunks of 128 partitions, H*W=1024 pixels
  on the free dim, processed per (sample, timestep) image.
* All convs on the TensorEngine in bf16:
    - 1x1 convs: plain matmuls, BN scales folded into weights on host.
    - depthwise 3x3: 9 accumulated matmuls with *diagonal* weight matrices
      and free-dim-shifted rhs access patterns into a padded tile.
* BN biases: pad tile border stays 0; all bias terms collapse analytically
  into a single per-channel bias added at the next LIF input (host-computed).
* LIF scans (4x) unrolled over T in fp32. Each step is 2 Vector-engine fused
  ops (scalar_tensor_tensor reading PSUM directly) + 1 compare op on
  GPSIMD/ScalarE. Spike outputs are written as bf16 (exact for 0/1) to feed
  the next matmul. The qk spatial sum rides the ScalarE Sign activation's
  accum_out for free.

bf16 matmul precision is safe here: reference final-LIF preactivations peak
at ~0.75 vs threshold 1.0 (verified numerically), so no spike flips occur.
"""
import sys
if "/opt/trn_rl_repo" not in sys.path:
    sys.path.insert(0, "/opt/trn_rl_repo")

import numpy as np
import ml_dtypes

from contextlib import ExitStack

import concourse.bacc as bacc
import concourse.tile as tile
from concourse import mybir
from concourse.bass_utils import run_bass_kernel_spmd

f32 = mybir.dt.float32
bf16 = mybir.dt.bfloat16
Alu = mybir.AluOpType
Act = mybir.ActivationFunctionType

EPS = 1e-5
T, B, C, H, W = 4, 16, 384, 32, 32
HW = H * W                    # 1024
KC = C // 128                 # 3 channel chunks
HP = H + 2                    # 34
PADF = HP * HP                # 1156
NCORES = 8
BL = B // NCORES              # 2 samples per core

bf = ml_dtypes.bfloat16


# --------------------------------------------------------------------------
# host-side weight preparation (pure numpy)
# --------------------------------------------------------------------------

def _affine(p):
    """BN params [4, c] -> (scale, bias) of the equivalent y = a*x + b."""
    w, b, m, v = np.asarray(p, np.float64)
    inv = w / np.sqrt(v + EPS)
    return (inv).astype(np.float32), (b - m * inv).astype(np.float32)


def _lhsT(wm):
    """[M, K] fp32 -> lhsT tile layout [128, KC, M] bf16 (k = kc*128+kp)."""
    k_m = np.ascontiguousarray(wm.T)                      # [K, M]
    return k_m.reshape(KC, 128, wm.shape[0]).transpose(1, 0, 2).astype(bf)


def _diag(dwt):
    """dw taps [C, 3, 3] -> diag lhsT tiles [128, KC, 9, 128] bf16."""
    out = np.zeros((128, KC, 9, 128), np.float32)
    taps = dwt.reshape(C, 9)                              # [c, tap]
    for kc in range(KC):
        for tap in range(9):
            out[np.arange(128), kc, tap, np.arange(128)] = \
                taps[kc * 128:(kc + 1) * 128, tap]
    return out.astype(bf)


def _cols(vec):
    """[C] -> per-partition column layout [128, KC] (c = kc*128 + kp)."""
    return np.ascontiguousarray(np.asarray(vec, np.float32).reshape(KC, 128).T)


def host_prep(r1_w1, r1_bn1, r1_dw, r1_pw, r1_bn2, qkv_bn,
              r2_w1, r2_bn1, r2_dw, r2_pw, r2_bn2, proj_bn):
    a1, b1 = _affine(r1_bn1)
    a2, b2 = _affine(r1_bn2)
    aq, bq = _affine(qkv_bn)
    a3, b3 = _affine(r2_bn1)
    a4, b4 = _affine(r2_bn2)
    ap_, bp = _affine(proj_bn)

    w1 = np.asarray(r1_w1, np.float32).reshape(C, C)
    pw = np.asarray(r1_pw, np.float32).reshape(2 * C, C)
    w2 = np.asarray(r2_w1, np.float32).reshape(C, C)
    pw2 = np.asarray(r2_pw, np.float32).reshape(C, C)
    dw1 = np.asarray(r1_dw, np.float32).reshape(C, 3, 3)
    dw2 = np.asarray(r2_dw, np.float32).reshape(C, 3, 3)

    # fold BN scales into conv weights (rows = output channels)
    w1f = a1[:, None] * w1                  # conv1 + bn1 scale
    A2 = aq * a2                            # bn2 o qkv_bn composed scale
    B2 = aq * b2 + bq
    pwf = A2[:, None] * pw
    w2f = a3[:, None] * w2
    A4 = ap_ * a4
    B4 = ap_ * b4 + bp
    pw2f = A4[:, None] * pw2

    # pad-border bias correction: true pad = our pad + b1 everywhere
    D1 = b1 * dw1.reshape(C, 9).sum(1)
    bias2 = B2 + pwf @ D1                   # [2C] bias at qk/v LIF input
    D2 = b3 * dw2.reshape(C, 9).sum(1)
    bias4 = B4 + pw2f @ D2                  # [C] bias at proj LIF input

    bqk, bv = bias2[:C], bias2[C:]
    cols = np.concatenate([
        _cols(bqk),            # 0:3   t=0 qk bias
        _cols(1 - 2 * bqk),    # 3:6   qk state const
        _cols(bv),             # 6:9
        _cols(1 - 2 * bv),     # 9:12
        _cols(bias4),          # 12:15
        _cols(1 - 2 * bias4),  # 15:18
        np.full((128, 1), -2.0, np.float32),  # 18: Sign bias
    ], axis=1)

    return dict(
        w1T=_lhsT(w1f), pwT=_lhsT(pwf), r2w1T=_lhsT(w2f), r2pwT=_lhsT(pw2f),
        diag1=_diag(dw1), diag2=_diag(dw2), cols=cols,
    )


# --------------------------------------------------------------------------
# device program
# --------------------------------------------------------------------------

def build(sc, repeat=1):
    """Build the per-core Bass program. sc = output scale (0.1)."""
    nc = bacc.Bacc("TRN2", target_bir_lowering=False, debug=False,
                   num_devices=NCORES)
    xin = nc.dram_tensor("xs", [T, BL, C, HW], f32, kind="ExternalInput").ap()
    w1T_d = nc.dram_tensor("w1T", [128, KC, C], bf16, kind="ExternalInput").ap()
    pwT_d = nc.dram_tensor("pwT", [128, KC, 2 * C], bf16, kind="ExternalInput").ap()
    r2w1T_d = nc.dram_tensor("r2w1T", [128, KC, C], bf16, kind="ExternalInput").ap()
    r2pwT_d = nc.dram_tensor("r2pwT", [128, KC, C], bf16, kind="ExternalInput").ap()
    diag1_d = nc.dram_tensor("diag1", [128, KC, 9, 128], bf16, kind="ExternalInput").ap()
    diag2_d = nc.dram_tensor("diag2", [128, KC, 9, 128], bf16, kind="ExternalInput").ap()
    cols_d = nc.dram_tensor("cols", [128, 19], f32, kind="ExternalInput").ap()
    out_d = nc.dram_tensor("out", [T, BL, C, HW], f32, kind="ExternalOutput").ap()

    with tile.TileContext(nc) as tc, ExitStack() as es:
        consts = es.enter_context(tc.tile_pool(name="consts", bufs=1))
        states = es.enter_context(tc.tile_pool(name="states", bufs=1))
        xp = es.enter_context(tc.tile_pool(name="xp", bufs=2))
        u1p = es.enter_context(tc.tile_pool(name="u1p", bufs=1))
        m1p = es.enter_context(tc.tile_pool(name="m1p", bufs=1))
        s1p = es.enter_context(tc.tile_pool(name="s1p", bufs=2))
        dwop = es.enter_context(tc.tile_pool(name="dwop", bufs=1))
        mskp = es.enter_context(tc.tile_pool(name="mskp", bufs=1))
        sv2p = es.enter_context(tc.tile_pool(name="sv2p", bufs=4))
        ump = es.enter_context(tc.tile_pool(name="ump", bufs=2))
        gp = es.enter_context(tc.tile_pool(name="gp", bufs=2))
        outp = es.enter_context(tc.tile_pool(name="outp", bufs=2))
        tinyp = es.enter_context(tc.tile_pool(name="tinyp", bufs=4))
        psp = es.enter_context(tc.tile_pool(name="psp", bufs=4, space="PSUM"))

        # ---- constants (loaded once) ----
        w1T = consts.tile([128, KC, C], bf16)
        pwT = consts.tile([128, KC, 2 * C], bf16)
        r2w1T = consts.tile([128, KC, C], bf16)
        r2pwT = consts.tile([128, KC, C], bf16)
        diag1 = consts.tile([128, KC, 9, 128], bf16)
        diag2 = consts.tile([128, KC, 9, 128], bf16)
        cols = consts.tile([128, 19], f32)
        for dst, src in [(w1T, w1T_d), (pwT, pwT_d), (r2w1T, r2w1T_d),
                         (r2pwT, r2pwT_d), (diag1, diag1_d), (diag2, diag2_d),
                         (cols, cols_d)]:
            nc.sync.dma_start(out=dst, in_=src)
        BQ0, CQ1, BV0, CV1, B40, C41, NEG2 = 0, 3, 6, 9, 12, 15, 18

        def col(base, mc):
            return cols[:, base + mc:base + mc + 1]

        # padded tiles; border stays 0 forever
        pad1 = consts.tile([128, KC, PADF], bf16)
        pad2 = consts.tile([128, KC, PADF], bf16)
        nc.vector.memset(pad1, 0.0)
        nc.vector.memset(pad2, 0.0)

        # ---- persistent per-sample state ----
        q1 = states.tile([128, KC, HW], f32)   # lif1 membrane (post reset)
        Wq = states.tile([128, KC, HW], f32)   # qk-lif scaled state
        Wv = states.tile([128, KC, HW], f32)   # v-lif scaled state
        W4 = states.tile([128, KC, HW], f32)   # proj-lif scaled state
        vth = states.tile([128, KC], f32)      # talking-heads membrane

        def mm_block(ps_tile, lhsT_tile, rhs_tile, n_k=KC):
            """1x1-conv block: accumulate over kc for both 512-col halves."""
            for nh in range(2):
                for kci in range(n_k):
                    nc.tensor.matmul(
                        ps_tile[:, nh * 512:(nh + 1) * 512],
                        lhsT_tile[:, kci, :],
                        rhs_tile[:, kci, nh * 512:(nh + 1) * 512],
                        start=(kci == 0), stop=(kci == n_k - 1))

        def dw_block(ps_tile, diag_tile, pad_tile, mc):
            """depthwise 3x3 for chunk mc: 9 diag matmuls per 512-col half."""
            padv = pad_tile[:, mc].rearrange("p (h w) -> p h w", h=HP)
            for nh in range(2):
                for tap in range(9):
                    i, j = divmod(tap, 3)
                    rhs = padv[:, i + nh * 16: i + nh * 16 + 16, j:j + 32]
                    nc.tensor.matmul(
                        ps_tile[:, nh * 512:(nh + 1) * 512],
                        diag_tile[:, mc, tap, :], rhs,
                        start=(tap == 0), stop=(tap == 8))

        for rep in range(repeat):
            for b in range(BL):
                nc.vector.memset(vth, 0.0)
                for t in range(T):
                    last = (t == T - 1)
                    # ---------- load x, LIF1 (hard reset, th=1) ----------
                    xt = xp.tile([128, KC, HW], f32)
                    nc.sync.dma_start(
                        out=xt,
                        in_=xin[t, b].rearrange("(kc kp) f -> kp kc f", kp=128))
                    if t == 0:
                        u1 = xt
                    else:
                        u1 = u1p.tile([128, KC, HW], f32)
                        nc.vector.tensor_add(u1, q1, xt)
                    s1 = s1p.tile([128, KC, HW], bf16)
                    nc.gpsimd.tensor_scalar(s1, u1, 2.0, None, Alu.is_ge)
                    if not last:
                        m1 = m1p.tile([128, KC, HW], bf16)
                        nc.gpsimd.tensor_scalar(m1, u1, 2.0, None, Alu.is_lt)
                        nc.vector.scalar_tensor_tensor(
                            q1, u1, 0.5, m1, Alu.mult, Alu.mult)

                    # ---------- repconv1: conv1 -> pad -> dw -> pw ----------
                    for mc in range(KC):
                        pc = psp.tile([128, HW], f32, tag="ps")
                        mm_block(pc, w1T[:, :, mc * 128:(mc + 1) * 128], s1)
                        nc.scalar.activation(
                            pad1[:, mc].rearrange("p (h w) -> p h w", h=HP)[:, 1:33, 1:33],
                            pc.rearrange("p (h w) -> p h w", h=32), Act.Copy)
                    dwo1 = dwop.tile([128, KC, HW], bf16)
                    for mc in range(KC):
                        pd = psp.tile([128, HW], f32, tag="ps")
                        dw_block(pd, diag1, pad1, mc)
                        nc.scalar.activation(dwo1[:, mc], pd, Act.Copy)

                    # ---------- pw1 (+qkv bn) fused into qk/v LIFs ----------
                    gsum = tinyp.tile([128, KC], f32)
                    sv2s = []
                    for mc in range(2 * KC):
                        pq = psp.tile([128, HW], f32, tag="ps")
                        mm_block(pq, pwT[:, :, mc * 128:(mc + 1) * 128], dwo1)
                        um = ump.tile([128, HW], f32)
                        if mc < KC:      # qk half: soft LIF, spatial sum
                            if t == 0:
                                nc.vector.tensor_scalar(um, pq, col(BQ0, mc),
                                                        None, Alu.add)
                            else:
                                nc.vector.scalar_tensor_tensor(
                                    um, Wq[:, mc], 0.5, pq, Alu.mult, Alu.add)
                            g2 = gp.tile([128, HW], bf16)
                            nc.scalar.activation(
                                g2, um, Act.Sign, bias=cols[:, NEG2:NEG2 + 1],
                                accum_out=gsum[:, mc:mc + 1])
                            if not last:
                                nc.vector.scalar_tensor_tensor(
                                    Wq[:, mc], um, col(CQ1, mc), g2,
                                    Alu.subtract, Alu.subtract)
                        else:            # v half: soft LIF, spike*2 kept
                            mv = mc - KC
                            if t == 0:
                                nc.vector.tensor_scalar(um, pq, col(BV0, mv),
                                                        None, Alu.add)
                            else:
                                nc.vector.scalar_tensor_tensor(
                                    um, Wv[:, mv], 0.5, pq, Alu.mult, Alu.add)
                            sv2 = sv2p.tile([128, HW], f32)
                            nc.gpsimd.tensor_scalar(sv2, um, 2.0, 2.0,
                                                    Alu.is_ge, Alu.mult)
                            sv2s.append(sv2)
                            if not last:
                                nc.vector.scalar_tensor_tensor(
                                    Wv[:, mv], um, col(CV1, mv), sv2,
                                    Alu.subtract, Alu.subtract)

                    # ---------- talking heads LIF (tiny, [128, KC]) ----------
                    uth = tinyp.tile([128, KC], f32)
                    nc.vector.scalar_tensor_tensor(uth, gsum, 0.5, vth,
                                                   Alu.mult, Alu.add)
                    qth = tinyp.tile([128, KC], f32)
                    nc.vector.tensor_scalar(qth, uth, -511.0, 0.5,
                                            Alu.is_ge, Alu.mult)
                    if not last:
                        mth = tinyp.tile([128, KC], f32)
                        nc.vector.tensor_scalar(mth, uth, -511.0, 0.5,
                                                Alu.is_lt, Alu.mult)
                        nc.vector.scalar_tensor_tensor(vth, uth, 512.0, mth,
                                                       Alu.add, Alu.mult)

                    # ---------- mask: out = qth * v_spike (binary) ----------
                    msk = mskp.tile([128, KC, HW], bf16)
                    for mv in range(KC):
                        nc.vector.tensor_scalar(msk[:, mv], sv2s[mv],
                                                qth[:, mv:mv + 1], None,
                                                Alu.mult)

                    # ---------- repconv2 ----------
                    for mc in range(KC):
                        pc = psp.tile([128, HW], f32, tag="ps")
                        mm_block(pc, r2w1T[:, :, mc * 128:(mc + 1) * 128], msk)
                        nc.scalar.activation(
                            pad2[:, mc].rearrange("p (h w) -> p h w", h=HP)[:, 1:33, 1:33],
                            pc.rearrange("p (h w) -> p h w", h=32), Act.Copy)
                    dwo2 = dwop.tile([128, KC, HW], bf16)
                    for mc in range(KC):
                        pd = psp.tile([128, HW], f32, tag="ps")
                        dw_block(pd, diag2, pad2, mc)
                        nc.scalar.activation(dwo2[:, mc], pd, Act.Copy)

                    # ---------- r2pw (+proj bn) fused into proj LIF ----------
                    for mc in range(KC):
                        pr = psp.tile([128, HW], f32, tag="ps")
                        mm_block(pr, r2pwT[:, :, mc * 128:(mc + 1) * 128], dwo2)
                        um = ump.tile([128, HW], f32)
                        if t == 0:
                            nc.vector.tensor_scalar(um, pr, col(B40, mc),
                                                    None, Alu.add)
                        else:
                            nc.vector.scalar_tensor_tensor(
                                um, W4[:, mc], 0.5, pr, Alu.mult, Alu.add)
                        g4 = gp.tile([128, HW], bf16)
                        nc.scalar.activation(g4, um, Act.Sign,
                                             bias=cols[:, NEG2:NEG2 + 1])
                        if not last:
                            nc.vector.scalar_tensor_tensor(
                                W4[:, mc], um, col(C41, mc), g4,
                                Alu.subtract, Alu.subtract)
                        ot = outp.tile([128, HW], f32)
                        nc.gpsimd.tensor_scalar(ot, g4, sc / 2, sc / 2,
                                                Alu.mult, Alu.add)
                        nc.sync.dma_start(
                            out=out_d[t, b].rearrange(
                                "(kc kp) f -> kp kc f", kp=128)[:, mc],
                            in_=ot)
    nc.finalize()
    return nc


_BUILD_CACHE = {}


def get_nc(sc, repeat=1):
    key = (float(sc), repeat)
    if key not in _BUILD_CACHE:
        _BUILD_CACHE[key] = build(float(sc), repeat)
    return _BUILD_CACHE[key]


def make_in_maps(inputs):
    x = np.asarray(inputs["x"], np.float32)
    prep = host_prep(**{k: inputs[k] for k in
                        ("r1_w1", "r1_bn1", "r1_dw", "r1_pw", "r1_bn2",
                         "qkv_bn", "r2_w1", "r2_bn1", "r2_dw", "r2_pw",
                         "r2_bn2", "proj_bn")})
    in_maps = []
    for i in range(NCORES):
        shard = np.ascontiguousarray(
            x[:, i * BL:(i + 1) * BL].reshape(T, BL, C, HW))
        in_maps.append({"xs": shard, **prep})
    return in_maps


def kernel(**inputs):
    sc = float(np.asarray(inputs["scale"]).reshape(-1)[0])
    nc = get_nc(sc)
    in_maps = make_in_maps(inputs)
    res = run_bass_kernel_spmd(nc, in_maps, core_ids=list(range(NCORES)))
    out = np.concatenate([res.results[i]["out"] for i in range(NCORES)],
                         axis=1)
    return out.reshape(T, B, C, H, W)


# revision 19
# speedup vs baseline: 3.1207x; 3.1207x over previous
"""Trainium2 Bass kernel for nn_MetaSDSA (spiking MetaFormer SDSA block).

Strategy
--------
* Data-parallel over batch: 8 cores x 2 samples each. Each core runs the full
  T=4 LIF recurrences for its samples, everything resident in SBUF.
* Channel-major layout: C=384 = 3 chunks of 128 partitions, H*W=1024 pixels
  on the free dim, processed per (sample, timestep) image.
* All convs on the TensorEngine in bf16:
    - 1x1 convs: plain matmuls, BN scales folded into weights on host.
    - depthwise 3x3: 9 accumulated matmuls with *diagonal* weight matrices
      and free-dim-shifted rhs access patterns into a padded tile.
* BN biases: pad tile border stays 0; all bias terms collapse analytically
  into a single per-channel bias added at the next LIF input (host-computed).
* LIF scans (4x) unrolled over T in fp32. Each step is 2 Vector-engine fused
  ops (scalar_tensor_tensor reading PSUM directly) + 1 compare op on
  GPSIMD/ScalarE. Spike outputs are written as bf16 (exact for 0/1) to feed
  the next matmul. The qk spatial sum rides the ScalarE Sign activation's
  accum_out for free.

bf16 matmul precision is safe here: reference final-LIF preactivations peak
at ~0.75 vs threshold 1.0 (verified numerically), so no spike flips occur.
"""
import sys
if "/opt/trn_rl_repo" not in sys.path:
    sys.path.insert(0, "/opt/trn_rl_repo")

import numpy as np
import ml_dtypes

from contextlib import ExitStack

import concourse.bacc as bacc
import concourse.tile as tile
from concourse import mybir
from concourse.bass_utils import run_bass_kernel_spmd

f32 = mybir.dt.float32
bf16 = mybir.dt.bfloat16
Alu = mybir.AluOpType
Act = mybir.ActivationFunctionType

EPS = 1e-5
T, B, C, H, W = 4, 16, 384, 32, 32
HW = H * W                    # 1024
KC = C // 128                 # 3 channel chunks
HP = H + 2                    # 34
PADF = HP * HP                # 1156
NCORES = 8
BL = B // NCORES              # 2 samples per core

bf = ml_dtypes.bfloat16


# --------------------------------------------------------------------------
# host-side weight preparation (pure numpy)
# --------------------------------------------------------------------------

def _affine(p):
    """BN params [4, c] -> (scale, bias) of the equivalent y = a*x + b."""
    w, b, m, v = np.asarray(p, np.float64)
    inv = w / np.sqrt(v + EPS)
    return (inv).astype(np.float32), (b - m * inv).astype(np.float32)


def _lhsT(wm):
    """[M, K] fp32 -> lhsT tile layout [128, KC, M] bf16 (k = kc*128+kp)."""
    k_m = np.ascontiguousarray(wm.T)                      # [K, M]
    return k_m.reshape(KC, 128, wm.shape[0]).transpose(1, 0, 2).astype(bf)


def _diag(dwt):
    """dw taps [C, 3, 3] -> diag lhsT tiles [128, KC, 9, 128] bf16."""
    out = np.zeros((128, KC, 9, 128), np.float32)
    taps = dwt.reshape(C, 9)                              # [c, tap]
    for kc in range(KC):
        for tap in range(9):
            out[np.arange(128), kc, tap, np.arange(128)] = \
                taps[kc * 128:(kc + 1) * 128, tap]
    return out.astype(bf)


def _cols(vec):
    """[C] -> per-partition column layout [128, KC] (c = kc*128 + kp)."""
    return np.ascontiguousarray(np.asarray(vec, np.float32).reshape(KC, 128).T)


def host_prep(r1_w1, r1_bn1, r1_dw, r1_pw, r1_bn2, qkv_bn,
              r2_w1, r2_bn1, r2_dw, r2_pw, r2_bn2, proj_bn):
    a1, b1 = _affine(r1_bn1)
    a2, b2 = _affine(r1_bn2)
    aq, bq = _affine(qkv_bn)
    a3, b3 = _affine(r2_bn1)
    a4, b4 = _affine(r2_bn2)
    ap_, bp = _affine(proj_bn)

    w1 = np.asarray(r1_w1, np.float32).reshape(C, C)
    pw = np.asarray(r1_pw, np.float32).reshape(2 * C, C)
    w2 = np.asarray(r2_w1, np.float32).reshape(C, C)
    pw2 = np.asarray(r2_pw, np.float32).reshape(C, C)
    dw1 = np.asarray(r1_dw, np.float32).reshape(C, 3, 3)
    dw2 = np.asarray(r2_dw, np.float32).reshape(C, 3, 3)

    # fold BN scales into conv weights (rows = output channels)
    w1f = a1[:, None] * w1                  # conv1 + bn1 scale
    A2 = aq * a2                            # bn2 o qkv_bn composed scale
    B2 = aq * b2 + bq
    pwf = A2[:, None] * pw
    w2f = a3[:, None] * w2
    A4 = ap_ * a4
    B4 = ap_ * b4 + bp
    pw2f = A4[:, None] * pw2

    # conv1 consumes the Sign tensor g1 = 2*s1 - 1: fold the /2 and the
    # +1/2 row-sum correction into weights and the downstream bias.
    w1g = w1f / 2
    c1 = w1g.sum(1)
    # pad-border bias correction: true pad = our pad + (b1 + c1) everywhere
    D1 = (b1 + c1) * dw1.reshape(C, 9).sum(1)
    bias2 = B2 + pwf @ D1                   # [2C] bias at qk/v LIF input
    D2 = b3 * dw2.reshape(C, 9).sum(1)
    bias4 = B4 + pw2f @ D2                  # [C] bias at proj LIF input

    bqk, bv = bias2[:C], bias2[C:]
    cols = np.concatenate([
        _cols(bqk),            # 0:3   t=0 qk bias
        _cols(1 - 2 * bqk),    # 3:6   qk state const (W = u - c - g)
        _cols(bv),             # 6:9
        _cols(1 - 2 * bv),     # 9:12
        _cols(bias4),          # 12:15
        _cols(1 - 2 * bias4),  # 15:18
        np.full((128, 1), -2.0, np.float32),  # 18: Sign bias
    ], axis=1)

    dw1r = dw1.reshape(C, 9).astype(bf).astype(np.float32)
    dw2r = dw2.reshape(C, 9).astype(bf).astype(np.float32)
    dwc = np.stack([
        np.stack([_cols(dw1r[:, tap]) for tap in range(9)], -1),
        np.stack([_cols(dw2r[:, tap]) for tap in range(9)], -1),
    ], 1)  # [128, 2, KC, 9]
    return dict(
        w1T=_lhsT(w1g), pwT=_lhsT(pwf), r2w1T=_lhsT(w2f), r2pwT=_lhsT(pw2f),
        diag1=_diag(dw1), diag2=_diag(dw2), cols=cols,
        dwc=np.ascontiguousarray(dwc, dtype=np.float32),
    )


# --------------------------------------------------------------------------
# device program
# --------------------------------------------------------------------------

def build(sc, repeat=1, dw_dve=(), pad_db=False, psum_fine=False,
          loop_repeat=None, boost=False):
    """Build the per-core Bass program. sc = output scale (0.1).

    dw_dve: set of (conv_idx, mc) whose depthwise chunk runs on the Vector
            engine (STT chain) instead of the TensorEngine.
    pad_db: double-buffer the padded tiles (alternate by timestep parity).
    """
    dw_dve = set(dw_dve)
    nc = bacc.Bacc("TRN2", target_bir_lowering=False, debug=False,
                   num_devices=NCORES)
    xin = nc.dram_tensor("xs", [T, BL, C, HW], f32, kind="ExternalInput").ap()
    w1T_d = nc.dram_tensor("w1T", [128, KC, C], bf16, kind="ExternalInput").ap()
    pwT_d = nc.dram_tensor("pwT", [128, KC, 2 * C], bf16, kind="ExternalInput").ap()
    r2w1T_d = nc.dram_tensor("r2w1T", [128, KC, C], bf16, kind="ExternalInput").ap()
    r2pwT_d = nc.dram_tensor("r2pwT", [128, KC, C], bf16, kind="ExternalInput").ap()
    diag1_d = nc.dram_tensor("diag1", [128, KC, 9, 128], bf16, kind="ExternalInput").ap()
    diag2_d = nc.dram_tensor("diag2", [128, KC, 9, 128], bf16, kind="ExternalInput").ap()
    cols_d = nc.dram_tensor("cols", [128, 19], f32, kind="ExternalInput").ap()
    dwc_d = nc.dram_tensor("dwc", [128, 2, KC, 9], f32, kind="ExternalInput").ap()
    out_d = nc.dram_tensor("out", [T, BL, C, HW], f32, kind="ExternalOutput").ap()

    with tile.TileContext(nc) as tc, ExitStack() as es:
        consts = es.enter_context(tc.tile_pool(name="consts", bufs=1))
        states = es.enter_context(tc.tile_pool(name="states", bufs=1))
        xp = es.enter_context(tc.tile_pool(name="xp", bufs=2))
        m1p = es.enter_context(tc.tile_pool(name="m1p", bufs=2))
        s1p = es.enter_context(tc.tile_pool(name="s1p", bufs=3 if boost else 2))
        dwo1p = es.enter_context(tc.tile_pool(name="dwo1p", bufs=3 if boost else 2))
        dwo2p = es.enter_context(tc.tile_pool(name="dwo2p", bufs=1))
        mskp = es.enter_context(tc.tile_pool(name="mskp", bufs=1))
        sv2p = es.enter_context(tc.tile_pool(name="sv2p", bufs=3))
        ump = es.enter_context(tc.tile_pool(name="ump", bufs=6 if boost else 4))
        gp = es.enter_context(tc.tile_pool(name="gp", bufs=6 if boost else 4))
        outp = es.enter_context(tc.tile_pool(name="outp", bufs=2))
        tinyp = es.enter_context(tc.tile_pool(name="tinyp", bufs=4))
        psp = es.enter_context(tc.tile_pool(name="psp", bufs=8, space="PSUM"))

        # ---- constants (loaded once) ----
        w1T = consts.tile([128, KC, C], bf16)
        pwT = consts.tile([128, KC, 2 * C], bf16)
        r2w1T = consts.tile([128, KC, C], bf16)
        r2pwT = consts.tile([128, KC, C], bf16)
        diag1 = consts.tile([128, KC, 9, 128], bf16)
        diag2 = consts.tile([128, KC, 9, 128], bf16)
        cols = consts.tile([128, 19], f32)
        dwc = consts.tile([128, 2, KC, 9], f32)
        for dst, srct in [(w1T, w1T_d), (pwT, pwT_d), (r2w1T, r2w1T_d),
                          (r2pwT, r2pwT_d), (diag1, diag1_d), (diag2, diag2_d),
                          (cols, cols_d), (dwc, dwc_d)]:
            nc.sync.dma_start(out=dst, in_=srct)
        BQ0, CQ1, BV0, CV1, B40, C41, NEG2 = 0, 3, 6, 9, 12, 15, 18

        def col(base, mc):
            return cols[:, base + mc:base + mc + 1]

        # padded tiles; border stays 0 forever
        npad = 2 if pad_db else 1
        pad1s = [consts.tile([128, KC, PADF], bf16, tag=f"pad1_{i}", name=f"pad1_{i}")
                 for i in range(npad)]
        pad2s = [consts.tile([128, KC, PADF], bf16, tag=f"pad2_{i}", name=f"pad2_{i}")
                 for i in range(npad)]
        for p in pad1s + pad2s:
            nc.vector.memset(p, 0.0)

        # ---- persistent per-sample state ----
        q1 = states.tile([128, KC, HW], f32)   # lif1 membrane (post reset)
        Wq = states.tile([128, KC, HW], f32)   # qk-lif scaled state
        Wv = states.tile([128, KC, HW], f32)   # v-lif scaled state
        W4 = states.tile([128, KC, HW], f32)   # proj-lif scaled state
        vth = states.tile([128, KC], f32)      # talking-heads membrane

        def mm_half(ps_tile, lhsT_tile, rhs_tile, nh, n_k=KC):
            """1x1-conv block: accumulate over kc for one 512-col half."""
            for kci in range(n_k):
                nc.tensor.matmul(
                    ps_tile,
                    lhsT_tile[:, kci, :],
                    rhs_tile[:, kci, nh * 512:(nh + 1) * 512],
                    start=(kci == 0), stop=(kci == n_k - 1))

        def dw_half(ps_tile, diag_tile, pad_tile, mc, nh):
            """depthwise 3x3, chunk mc, one 512-col half: 9 diag matmuls."""
            padv = pad_tile[:, mc].rearrange("p (h w) -> p h w", h=HP)
            for tap in range(9):
                i, j = divmod(tap, 3)
                rhs = padv[:, i + nh * 16: i + nh * 16 + 16, j:j + 32]
                nc.tensor.matmul(
                    ps_tile, diag_tile[:, mc, tap, :], rhs,
                    start=(tap == 0), stop=(tap == 8))

        dwaccp = es.enter_context(tc.tile_pool(name="dwaccp", bufs=1))

        def dw_block_dve(out_bf, conv_idx, pad_tile, mc):
            """depthwise 3x3 on the Vector engine: 9-tap STT MAC chain."""
            padv = pad_tile[:, mc].rearrange("p (h w) -> p h w", h=HP)
            acc = dwaccp.tile([128, HW], f32, tag="dwacc")
            accv = acc.rearrange("p (h w) -> p h w", h=32)
            for tap in range(9):
                i, j = divmod(tap, 3)
                rhs = padv[:, i:i + 32, j:j + 32]
                dcol = dwc[:, conv_idx, mc, tap:tap + 1]
                if tap == 0:
                    nc.vector.tensor_scalar(accv, rhs, dcol, None, Alu.mult)
                elif tap < 8:
                    nc.vector.scalar_tensor_tensor(accv, rhs, dcol, accv,
                                                   Alu.mult, Alu.add)
                else:
                    nc.vector.scalar_tensor_tensor(
                        out_bf.rearrange("p (h w) -> p h w", h=32), rhs, dcol,
                        accv, Alu.mult, Alu.add)

        def lif1_stage(b, t):
            """Load x[t,b] and run one LIF1 step, per 128-channel chunk.
            Returns the bf16 spike tile that feeds conv1."""
            last = (t == T - 1)
            xt = xp.tile([128, KC, HW], f32, tag="xt", name=f"xt_{b}_{t}")
            nc.sync.dma_start(
                out=xt,
                in_=xin[t, b].rearrange("(kc kp) f -> kp kc f", kp=128))
            s1 = s1p.tile([128, KC, HW], bf16, tag="s1", name=f"s1_{b}_{t}")
            for mc in range(KC):
                u1c = xt[:, mc]
                if t > 0:
                    nc.vector.tensor_add(u1c, q1[:, mc], xt[:, mc])
                nc.scalar.activation(s1[:, mc], u1c, Act.Sign,
                                     bias=cols[:, NEG2:NEG2 + 1])
                if not last:
                    m1 = m1p.tile([128, HW], bf16, tag="m1")
                    nc.vector.tensor_scalar(m1, s1[:, mc], -0.25, 0.25,
                                            Alu.mult, Alu.add)
                    nc.vector.tensor_mul(q1[:, mc], u1c, m1)
            return s1

        def conv1_stage(b, t, s1):
            """conv1 matmuls + pad1 interior epilogue for (b, t)."""
            pad1 = pad1s[t % npad]
            for mc in range(KC):
                padi = pad1[:, mc].rearrange(
                    "p (h w) -> p h w", h=HP)[:, 1:33, 1:33]
                for nh in range(2):
                    pc = psp.tile([128, 512], f32, tag="ps")
                    mm_half(pc, w1T[:, :, mc * 128:(mc + 1) * 128], s1, nh)
                    nc.scalar.activation(
                        padi[:, nh * 16:(nh + 1) * 16, :],
                        pc.rearrange("p (h w) -> p h w", h=16), Act.Copy)

        def dw1_stage(b, t):
            pad1 = pad1s[t % npad]
            dwo1 = dwo1p.tile([128, KC, HW], bf16, tag="dwo1",
                              name=f"dwo1_{b}_{t}")
            for mc in range(KC):
                if (0, mc) in dw_dve:
                    dw_block_dve(dwo1[:, mc], 0, pad1, mc)
                    continue
                for nh in range(2):
                    pd = psp.tile([128, 512], f32, tag="ps")
                    dw_half(pd, diag1, pad1, mc, nh)
                    nc.scalar.activation(
                        dwo1[:, mc, nh * 512:(nh + 1) * 512], pd, Act.Copy)
            return dwo1

        def pw1_lif_stage(b, t, dwo1):
            last = (t == T - 1)
            gsum = tinyp.tile([128, KC, 2], f32, tag="gsum")
            sv2s = []
            for mc in range(2 * KC):
                sv2 = None
                if mc >= KC:
                    sv2 = sv2p.tile([128, HW], bf16, tag="sv2")
                    sv2s.append(sv2)
                for nh in range(2):
                    hsl = slice(nh * 512, (nh + 1) * 512)
                    pq = psp.tile([128, 512], f32, tag="ps")
                    mm_half(pq, pwT[:, :, mc * 128:(mc + 1) * 128], dwo1, nh)
                    um = ump.tile([128, 512], f32, tag="um")
                    if mc < KC:      # qk half: soft LIF, spatial sum
                        if t == 0:
                            nc.vector.tensor_scalar(
                                um, pq, col(BQ0, mc), None, Alu.add)
                        else:
                            nc.vector.scalar_tensor_tensor(
                                um, Wq[:, mc, hsl], 0.5, pq,
                                Alu.mult, Alu.add)
                        g2 = gp.tile([128, 512], bf16, tag="g")
                        nc.scalar.activation(
                            g2, um, Act.Sign, bias=cols[:, NEG2:NEG2 + 1],
                            accum_out=gsum[:, mc, nh:nh + 1])
                        if not last:
                            nc.vector.scalar_tensor_tensor(
                                Wq[:, mc, hsl], um, col(CQ1, mc), g2,
                                Alu.subtract, Alu.subtract)
                    else:            # v half: soft LIF, spike*2 kept
                        mv = mc - KC
                        if t == 0:
                            nc.vector.tensor_scalar(
                                um, pq, col(BV0, mv), None, Alu.add)
                        else:
                            nc.vector.scalar_tensor_tensor(
                                um, Wv[:, mv, hsl], 0.5, pq,
                                Alu.mult, Alu.add)
                        nc.scalar.activation(sv2[:, hsl], um, Act.Sign,
                                             bias=cols[:, NEG2:NEG2 + 1])
                        if not last:
                            nc.vector.scalar_tensor_tensor(
                                Wv[:, mv, hsl], um, col(CV1, mv), sv2[:, hsl],
                                Alu.subtract, Alu.subtract)
            return gsum, sv2s

        def th_mask_stage(b, t, gsum, sv2s):
            last = (t == T - 1)
            gsum2 = tinyp.tile([128, KC], f32, tag="gsum2")
            nc.vector.tensor_add(gsum2, gsum[:, :, 0], gsum[:, :, 1])
            uth = tinyp.tile([128, KC], f32)
            nc.vector.scalar_tensor_tensor(uth, gsum2, 0.5, vth,
                                           Alu.mult, Alu.add)
            qth = tinyp.tile([128, KC], f32)
            nc.vector.tensor_scalar(qth, uth, -511.0, 0.5,
                                    Alu.is_ge, Alu.mult)
            if not last:
                mth = tinyp.tile([128, KC], f32)
                nc.vector.tensor_scalar(mth, uth, -511.0, 0.5,
                                        Alu.is_lt, Alu.mult)
                nc.vector.scalar_tensor_tensor(vth, uth, 512.0, mth,
                                               Alu.add, Alu.mult)
            # msk = spike * qth01 = g3*(qth01/2) + qth01/2, qth in {0, 0.5}
            msk = mskp.tile([128, KC, HW], bf16)
            for mv in range(KC):
                nc.vector.tensor_scalar(msk[:, mv], sv2s[mv],
                                        qth[:, mv:mv + 1],
                                        qth[:, mv:mv + 1],
                                        Alu.mult, Alu.add)
            return msk

        def tail_stage(b, t, msk):
            last = (t == T - 1)
            pad2 = pad2s[t % npad]
            for mc in range(KC):
                padi = pad2[:, mc].rearrange(
                    "p (h w) -> p h w", h=HP)[:, 1:33, 1:33]
                for nh in range(2):
                    pc = psp.tile([128, 512], f32, tag="ps")
                    mm_half(pc, r2w1T[:, :, mc * 128:(mc + 1) * 128], msk, nh)
                    nc.scalar.activation(
                        padi[:, nh * 16:(nh + 1) * 16, :],
                        pc.rearrange("p (h w) -> p h w", h=16), Act.Copy)
            dwo2 = dwo2p.tile([128, KC, HW], bf16, tag="dwo2")
            for mc in range(KC):
                if (1, mc) in dw_dve:
                    dw_block_dve(dwo2[:, mc], 1, pad2, mc)
                    continue
                for nh in range(2):
                    pd = psp.tile([128, 512], f32, tag="ps")
                    dw_half(pd, diag2, pad2, mc, nh)
                    nc.scalar.activation(
                        dwo2[:, mc, nh * 512:(nh + 1) * 512], pd, Act.Copy)
            for mc in range(KC):
                for nh in range(2):
                    hsl = slice(nh * 512, (nh + 1) * 512)
                    pr = psp.tile([128, 512], f32, tag="ps")
                    mm_half(pr, r2pwT[:, :, mc * 128:(mc + 1) * 128], dwo2, nh)
                    um = ump.tile([128, 512], f32, tag="um")
                    if t == 0:
                        nc.vector.tensor_scalar(
                            um, pr, col(B40, mc), None, Alu.add)
                    else:
                        nc.vector.scalar_tensor_tensor(
                            um, W4[:, mc, hsl], 0.5, pr, Alu.mult, Alu.add)
                    g4 = gp.tile([128, 512], bf16, tag="g")
                    nc.scalar.activation(g4, um, Act.Sign,
                                         bias=cols[:, NEG2:NEG2 + 1])
                    if not last:
                        nc.vector.scalar_tensor_tensor(
                            W4[:, mc, hsl], um, col(C41, mc), g4,
                            Alu.subtract, Alu.subtract)
                    ot = outp.tile([128, 512], f32, tag="ot")
                    nc.vector.tensor_scalar(ot, g4, sc / 2, sc / 2,
                                            Alu.mult, Alu.add)
                    nc.sync.dma_start(
                        out=out_d[t, b].rearrange(
                            "(kc kp) f -> kp kc f", kp=128)[:, mc, hsl],
                        in_=ot)

        import contextlib
        loop_cm = (tc.For_i(0, loop_repeat, 1) if loop_repeat
                   else contextlib.nullcontext())
        with loop_cm:
          for rep in range(repeat):
            for b in range(BL):
                nc.vector.memset(vth, 0.0)
                # prologue: lif1/conv1/dw1 for t=0
                s1 = lif1_stage(b, 0)
                conv1_stage(b, 0, s1)
                dwo1 = dw1_stage(b, 0)
                for t in range(T):
                    last = (t == T - 1)
                    gsum, sv2s = pw1_lif_stage(b, t, dwo1)
                    if not last:
                        s1 = lif1_stage(b, t + 1)
                        conv1_stage(b, t + 1, s1)
                    msk = th_mask_stage(b, t, gsum, sv2s)
                    if not last:
                        dwo1 = dw1_stage(b, t + 1)
                    tail_stage(b, t, msk)
    nc.finalize()
    return nc


_BUILD_CACHE = {}


def get_nc(sc, repeat=1, **kw):
    key = (float(sc), repeat, tuple(sorted(kw.items())))
    if key not in _BUILD_CACHE:
        _BUILD_CACHE[key] = build(float(sc), repeat, **kw)
    return _BUILD_CACHE[key]


def make_in_maps(inputs):
    x = np.asarray(inputs["x"], np.float32)
    prep = host_prep(**{k: inputs[k] for k in
                        ("r1_w1", "r1_bn1", "r1_dw", "r1_pw", "r1_bn2",
                         "qkv_bn", "r2_w1", "r2_bn1", "r2_dw", "r2_pw",
                         "r2_bn2", "proj_bn")})
    in_maps = []
    for i in range(NCORES):
        shard = np.ascontiguousarray(
            x[:, i * BL:(i + 1) * BL].reshape(T, BL, C, HW))
        in_maps.append({"xs": shard, **prep})
    return in_maps


def kernel(**inputs):
    sc = float(np.asarray(inputs["scale"]).reshape(-1)[0])
    nc = get_nc(sc, pad_db=True)
    in_maps = make_in_maps(inputs)
    res = run_bass_kernel_spmd(nc, in_maps, core_ids=list(range(NCORES)))
    out = np.concatenate([res.results[i]["out"] for i in range(NCORES)],
                         axis=1)
    return out.reshape(T, B, C, H, W)


# revision 20
# speedup vs baseline: 3.7486x; 1.2012x over previous
"""Trainium2 Bass kernel for nn_MetaSDSA (spiking MetaFormer SDSA block).

Strategy
--------
* Data-parallel over batch: 8 cores x 2 samples each. Each core runs the full
  T=4 LIF recurrences for its samples, everything resident in SBUF.
* Channel-major layout: C=384 = 3 chunks of 128 partitions, H*W=1024 pixels
  on the free dim, processed per (sample, timestep) image.
* All convs on the TensorEngine in bf16:
    - 1x1 convs: plain matmuls, BN scales folded into weights on host.
    - depthwise 3x3: 9 accumulated matmuls with *diagonal* weight matrices
      and free-dim-shifted rhs access patterns into a padded tile.
* BN biases: pad tile border stays 0; all bias terms collapse analytically
  into a single per-channel bias added at the next LIF input (host-computed).
* LIF scans (4x) unrolled over T in fp32. Each step is 2 Vector-engine fused
  ops (scalar_tensor_tensor reading PSUM directly) + 1 compare op on
  GPSIMD/ScalarE. Spike outputs are written as bf16 (exact for 0/1) to feed
  the next matmul. The qk spatial sum rides the ScalarE Sign activation's
  accum_out for free.

bf16 matmul precision is safe here: reference final-LIF preactivations peak
at ~0.75 vs threshold 1.0 (verified numerically), so no spike flips occur.
"""
import sys
if "/opt/trn_rl_repo" not in sys.path:
    sys.path.insert(0, "/opt/trn_rl_repo")

import numpy as np
import ml_dtypes

from contextlib import ExitStack

import concourse.bacc as bacc
import concourse.tile as tile
from concourse import mybir
from concourse.bass_utils import run_bass_kernel_spmd

f32 = mybir.dt.float32
bf16 = mybir.dt.bfloat16
Alu = mybir.AluOpType
Act = mybir.ActivationFunctionType

EPS = 1e-5
T, B, C, H, W = 4, 16, 384, 32, 32
HW = H * W                    # 1024
KC = C // 128                 # 3 channel chunks
HP = H + 2                    # 34
PADF = HP * HP                # 1156
NCORES = 8
BL = B // NCORES              # 2 samples per core

bf = ml_dtypes.bfloat16


# --------------------------------------------------------------------------
# host-side weight preparation (pure numpy)
# --------------------------------------------------------------------------

def _affine(p):
    """BN params [4, c] -> (scale, bias) of the equivalent y = a*x + b."""
    w, b, m, v = np.asarray(p, np.float64)
    inv = w / np.sqrt(v + EPS)
    return (inv).astype(np.float32), (b - m * inv).astype(np.float32)


def _lhsT(wm):
    """[M, K] fp32 -> lhsT tile layout [128, KC, M] bf16 (k = kc*128+kp)."""
    k_m = np.ascontiguousarray(wm.T)                      # [K, M]
    return k_m.reshape(KC, 128, wm.shape[0]).transpose(1, 0, 2).astype(bf)


def _diag(dwt):
    """dw taps [C, 3, 3] -> diag lhsT tiles [128, KC, 9, 128] bf16."""
    out = np.zeros((128, KC, 9, 128), np.float32)
    taps = dwt.reshape(C, 9)                              # [c, tap]
    for kc in range(KC):
        for tap in range(9):
            out[np.arange(128), kc, tap, np.arange(128)] = \
                taps[kc * 128:(kc + 1) * 128, tap]
    return out.astype(bf)


def _cols(vec):
    """[C] -> per-partition column layout [128, KC] (c = kc*128 + kp)."""
    return np.ascontiguousarray(np.asarray(vec, np.float32).reshape(KC, 128).T)


def host_prep(r1_w1, r1_bn1, r1_dw, r1_pw, r1_bn2, qkv_bn,
              r2_w1, r2_bn1, r2_dw, r2_pw, r2_bn2, proj_bn):
    a1, b1 = _affine(r1_bn1)
    a2, b2 = _affine(r1_bn2)
    aq, bq = _affine(qkv_bn)
    a3, b3 = _affine(r2_bn1)
    a4, b4 = _affine(r2_bn2)
    ap_, bp = _affine(proj_bn)

    w1 = np.asarray(r1_w1, np.float32).reshape(C, C)
    pw = np.asarray(r1_pw, np.float32).reshape(2 * C, C)
    w2 = np.asarray(r2_w1, np.float32).reshape(C, C)
    pw2 = np.asarray(r2_pw, np.float32).reshape(C, C)
    dw1 = np.asarray(r1_dw, np.float32).reshape(C, 3, 3)
    dw2 = np.asarray(r2_dw, np.float32).reshape(C, 3, 3)

    # fold BN scales into conv weights (rows = output channels)
    w1f = a1[:, None] * w1                  # conv1 + bn1 scale
    A2 = aq * a2                            # bn2 o qkv_bn composed scale
    B2 = aq * b2 + bq
    pwf = A2[:, None] * pw
    w2f = a3[:, None] * w2
    A4 = ap_ * a4
    B4 = ap_ * b4 + bp
    pw2f = A4[:, None] * pw2

    # conv1 consumes the Sign tensor g1 = 2*s1 - 1: fold the /2 and the
    # +1/2 row-sum correction into weights and the downstream bias.
    w1g = w1f / 2
    c1 = w1g.sum(1)
    # pad-border bias correction: true pad = our pad + (b1 + c1) everywhere
    D1 = (b1 + c1) * dw1.reshape(C, 9).sum(1)
    bias2 = B2 + pwf @ D1                   # [2C] bias at qk/v LIF input
    D2 = b3 * dw2.reshape(C, 9).sum(1)
    bias4 = B4 + pw2f @ D2                  # [C] bias at proj LIF input

    bqk, bv = bias2[:C], bias2[C:]
    cols = np.concatenate([
        _cols(bqk),            # 0:3   t=0 qk bias
        _cols(1 - 2 * bqk),    # 3:6   qk state const (W = u - c - g)
        _cols(bv),             # 6:9
        _cols(1 - 2 * bv),     # 9:12
        _cols(bias4),          # 12:15
        _cols(1 - 2 * bias4),  # 15:18
        np.full((128, 1), -2.0, np.float32),  # 18: Sign bias
    ], axis=1)

    dw1r = dw1.reshape(C, 9).astype(bf).astype(np.float32)
    dw2r = dw2.reshape(C, 9).astype(bf).astype(np.float32)
    dwc = np.stack([
        np.stack([_cols(dw1r[:, tap]) for tap in range(9)], -1),
        np.stack([_cols(dw2r[:, tap]) for tap in range(9)], -1),
    ], 1)  # [128, 2, KC, 9]
    return dict(
        w1T=_lhsT(w1g), pwT=_lhsT(pwf), r2w1T=_lhsT(w2f), r2pwT=_lhsT(pw2f),
        diag1=_diag(dw1), diag2=_diag(dw2), cols=cols,
        dwc=np.ascontiguousarray(dwc, dtype=np.float32),
    )


# --------------------------------------------------------------------------
# device program
# --------------------------------------------------------------------------

def build(sc, repeat=1, dw_dve=(), pad_db=False, psum_fine=False,
          loop_repeat=None, boost=False):
    """Build the per-core Bass program. sc = output scale (0.1).

    dw_dve: set of (conv_idx, mc) whose depthwise chunk runs on the Vector
            engine (STT chain) instead of the TensorEngine.
    pad_db: double-buffer the padded tiles (alternate by timestep parity).
    """
    dw_dve = set(dw_dve)
    nc = bacc.Bacc("TRN2", target_bir_lowering=False, debug=False,
                   num_devices=NCORES)
    xin = nc.dram_tensor("xs", [T, BL, C, HW], f32, kind="ExternalInput").ap()
    w1T_d = nc.dram_tensor("w1T", [128, KC, C], bf16, kind="ExternalInput").ap()
    pwT_d = nc.dram_tensor("pwT", [128, KC, 2 * C], bf16, kind="ExternalInput").ap()
    r2w1T_d = nc.dram_tensor("r2w1T", [128, KC, C], bf16, kind="ExternalInput").ap()
    r2pwT_d = nc.dram_tensor("r2pwT", [128, KC, C], bf16, kind="ExternalInput").ap()
    diag1_d = nc.dram_tensor("diag1", [128, KC, 9, 128], bf16, kind="ExternalInput").ap()
    diag2_d = nc.dram_tensor("diag2", [128, KC, 9, 128], bf16, kind="ExternalInput").ap()
    cols_d = nc.dram_tensor("cols", [128, 19], f32, kind="ExternalInput").ap()
    dwc_d = nc.dram_tensor("dwc", [128, 2, KC, 9], f32, kind="ExternalInput").ap()
    out_d = nc.dram_tensor("out", [T, BL, C, HW], f32, kind="ExternalOutput").ap()

    with tile.TileContext(nc) as tc, ExitStack() as es:
        consts = es.enter_context(tc.tile_pool(name="consts", bufs=1))
        states = es.enter_context(tc.tile_pool(name="states", bufs=1))
        xp = es.enter_context(tc.tile_pool(name="xp", bufs=2))
        m1p = es.enter_context(tc.tile_pool(name="m1p", bufs=2))
        s1p = es.enter_context(tc.tile_pool(name="s1p", bufs=3 if boost else 2))
        dwo1p = es.enter_context(tc.tile_pool(name="dwo1p", bufs=3 if boost else 2))
        dwo2p = es.enter_context(tc.tile_pool(name="dwo2p", bufs=1))
        mskp = es.enter_context(tc.tile_pool(name="mskp", bufs=1))
        sv2p = es.enter_context(tc.tile_pool(name="sv2p", bufs=3))
        ump = es.enter_context(tc.tile_pool(name="ump", bufs=6 if boost else 4))
        gp = es.enter_context(tc.tile_pool(name="gp", bufs=6 if boost else 4))
        outp = es.enter_context(tc.tile_pool(name="outp", bufs=2))
        tinyp = es.enter_context(tc.tile_pool(name="tinyp", bufs=4))
        psp = es.enter_context(tc.tile_pool(name="psp", bufs=8, space="PSUM"))

        # ---- constants (loaded once) ----
        w1T = consts.tile([128, KC, C], bf16)
        pwT = consts.tile([128, KC, 2 * C], bf16)
        r2w1T = consts.tile([128, KC, C], bf16)
        r2pwT = consts.tile([128, KC, C], bf16)
        diag1 = consts.tile([128, KC, 9, 128], bf16)
        diag2 = consts.tile([128, KC, 9, 128], bf16)
        cols = consts.tile([128, 19], f32)
        dwc = consts.tile([128, 2, KC, 9], f32)
        for dst, srct in [(w1T, w1T_d), (pwT, pwT_d), (r2w1T, r2w1T_d),
                          (r2pwT, r2pwT_d), (diag1, diag1_d), (diag2, diag2_d),
                          (cols, cols_d), (dwc, dwc_d)]:
            nc.sync.dma_start(out=dst, in_=srct)
        BQ0, CQ1, BV0, CV1, B40, C41, NEG2 = 0, 3, 6, 9, 12, 15, 18

        def col(base, mc):
            return cols[:, base + mc:base + mc + 1]

        # padded tiles; border stays 0 forever
        npad = 2 if pad_db else 1
        pad1s = [consts.tile([128, KC, PADF], bf16, tag=f"pad1_{i}", name=f"pad1_{i}")
                 for i in range(npad)]
        pad2s = [consts.tile([128, KC, PADF], bf16, tag=f"pad2_{i}", name=f"pad2_{i}")
                 for i in range(npad)]
        for p in pad1s + pad2s:
            nc.vector.memset(p, 0.0)

        # ---- persistent per-sample state ----
        q1 = states.tile([128, KC, HW], f32)   # lif1 membrane (post reset)
        Wq = states.tile([128, KC, HW], f32)   # qk-lif scaled state
        Wv = states.tile([128, KC, HW], f32)   # v-lif scaled state
        W4 = states.tile([128, KC, HW], f32)   # proj-lif scaled state
        vth = states.tile([128, KC], f32)      # talking-heads membrane

        def mm_half(ps_tile, lhsT_tile, rhs_tile, nh, n_k=KC):
            """1x1-conv block: accumulate over kc for one 512-col half."""
            for kci in range(n_k):
                nc.tensor.matmul(
                    ps_tile,
                    lhsT_tile[:, kci, :],
                    rhs_tile[:, kci, nh * 512:(nh + 1) * 512],
                    start=(kci == 0), stop=(kci == n_k - 1))

        def dw_half(ps_tile, diag_tile, pad_tile, mc, nh):
            """depthwise 3x3, chunk mc, one 512-col half: 9 diag matmuls."""
            padv = pad_tile[:, mc].rearrange("p (h w) -> p h w", h=HP)
            for tap in range(9):
                i, j = divmod(tap, 3)
                rhs = padv[:, i + nh * 16: i + nh * 16 + 16, j:j + 32]
                nc.tensor.matmul(
                    ps_tile, diag_tile[:, mc, tap, :], rhs,
                    start=(tap == 0), stop=(tap == 8))

        dwaccp = es.enter_context(tc.tile_pool(name="dwaccp", bufs=1))

        def dw_block_dve(out_bf, conv_idx, pad_tile, mc):
            """depthwise 3x3 on the Vector engine: 9-tap STT MAC chain."""
            padv = pad_tile[:, mc].rearrange("p (h w) -> p h w", h=HP)
            acc = dwaccp.tile([128, HW], f32, tag="dwacc")
            accv = acc.rearrange("p (h w) -> p h w", h=32)
            for tap in range(9):
                i, j = divmod(tap, 3)
                rhs = padv[:, i:i + 32, j:j + 32]
                dcol = dwc[:, conv_idx, mc, tap:tap + 1]
                if tap == 0:
                    nc.vector.tensor_scalar(accv, rhs, dcol, None, Alu.mult)
                elif tap < 8:
                    nc.vector.scalar_tensor_tensor(accv, rhs, dcol, accv,
                                                   Alu.mult, Alu.add)
                else:
                    nc.vector.scalar_tensor_tensor(
                        out_bf.rearrange("p (h w) -> p h w", h=32), rhs, dcol,
                        accv, Alu.mult, Alu.add)

        def lif1_stage(b, t):
            """Load x[t,b] and run one LIF1 step, per 128-channel chunk.
            Returns the bf16 spike tile that feeds conv1."""
            last = (t == T - 1)
            xt = xp.tile([128, KC, HW], f32, tag="xt", name=f"xt_{b}_{t}")
            nc.sync.dma_start(
                out=xt,
                in_=xin[t, b].rearrange("(kc kp) f -> kp kc f", kp=128))
            s1 = s1p.tile([128, KC, HW], bf16, tag="s1", name=f"s1_{b}_{t}")
            for mc in range(KC):
                u1c = xt[:, mc]
                if t > 0:
                    nc.vector.tensor_add(u1c, q1[:, mc], xt[:, mc])
                nc.scalar.activation(s1[:, mc], u1c, Act.Sign,
                                     bias=cols[:, NEG2:NEG2 + 1])
                if not last:
                    m1 = m1p.tile([128, HW], bf16, tag="m1")
                    nc.vector.tensor_scalar(m1, s1[:, mc], -0.25, 0.25,
                                            Alu.mult, Alu.add)
                    nc.vector.tensor_mul(q1[:, mc], u1c, m1)
            return s1

        def conv1_stage(b, t, s1):
            """conv1 matmuls + pad1 interior epilogue for (b, t)."""
            pad1 = pad1s[t % npad]
            for mc in range(KC):
                padi = pad1[:, mc].rearrange(
                    "p (h w) -> p h w", h=HP)[:, 1:33, 1:33]
                for nh in range(2):
                    pc = psp.tile([128, 512], f32, tag="ps")
                    mm_half(pc, w1T[:, :, mc * 128:(mc + 1) * 128], s1, nh)
                    nc.scalar.activation(
                        padi[:, nh * 16:(nh + 1) * 16, :],
                        pc.rearrange("p (h w) -> p h w", h=16), Act.Copy)

        def dw1_stage(b, t):
            pad1 = pad1s[t % npad]
            dwo1 = dwo1p.tile([128, KC, HW], bf16, tag="dwo1",
                              name=f"dwo1_{b}_{t}")
            for mc in range(KC):
                if (0, mc) in dw_dve:
                    dw_block_dve(dwo1[:, mc], 0, pad1, mc)
                    continue
                for nh in range(2):
                    pd = psp.tile([128, 512], f32, tag="ps")
                    dw_half(pd, diag1, pad1, mc, nh)
                    nc.scalar.activation(
                        dwo1[:, mc, nh * 512:(nh + 1) * 512], pd, Act.Copy)
            return dwo1

        def pw1_lif_stage(b, t, dwo1):
            last = (t == T - 1)
            gsum = tinyp.tile([128, KC, 2], f32, tag="gsum")
            sv2s = []
            for mc in range(2 * KC):
                sv2 = None
                if mc >= KC:
                    sv2 = sv2p.tile([128, HW], bf16, tag="sv2")
                    sv2s.append(sv2)
                for nh in range(2):
                    hsl = slice(nh * 512, (nh + 1) * 512)
                    pq = psp.tile([128, 512], f32, tag="ps")
                    mm_half(pq, pwT[:, :, mc * 128:(mc + 1) * 128], dwo1, nh)
                    um = ump.tile([128, 512], f32, tag="um")
                    if mc < KC:      # qk half: soft LIF, spatial sum
                        if t == 0:
                            nc.vector.tensor_scalar(
                                um, pq, col(BQ0, mc), None, Alu.add)
                        else:
                            nc.vector.scalar_tensor_tensor(
                                um, Wq[:, mc, hsl], 0.5, pq,
                                Alu.mult, Alu.add)
                        g2 = gp.tile([128, 512], bf16, tag="g")
                        nc.scalar.activation(
                            g2, um, Act.Sign, bias=cols[:, NEG2:NEG2 + 1],
                            accum_out=gsum[:, mc, nh:nh + 1])
                        if not last:
                            nc.vector.scalar_tensor_tensor(
                                Wq[:, mc, hsl], um, col(CQ1, mc), g2,
                                Alu.subtract, Alu.subtract)
                    else:            # v half: soft LIF, spike*2 kept
                        mv = mc - KC
                        if t == 0:
                            nc.vector.tensor_scalar(
                                um, pq, col(BV0, mv), None, Alu.add)
                        else:
                            nc.vector.scalar_tensor_tensor(
                                um, Wv[:, mv, hsl], 0.5, pq,
                                Alu.mult, Alu.add)
                        nc.scalar.activation(sv2[:, hsl], um, Act.Sign,
                                             bias=cols[:, NEG2:NEG2 + 1])
                        if not last:
                            nc.vector.scalar_tensor_tensor(
                                Wv[:, mv, hsl], um, col(CV1, mv), sv2[:, hsl],
                                Alu.subtract, Alu.subtract)
            return gsum, sv2s

        def th_mask_stage(b, t, gsum, sv2s):
            last = (t == T - 1)
            if t == 0:
                nc.vector.memset(vth, 0.0)
            gsum2 = tinyp.tile([128, KC], f32, tag="gsum2")
            nc.vector.tensor_add(gsum2, gsum[:, :, 0], gsum[:, :, 1])
            uth = tinyp.tile([128, KC], f32)
            nc.vector.scalar_tensor_tensor(uth, gsum2, 0.5, vth,
                                           Alu.mult, Alu.add)
            qth = tinyp.tile([128, KC], f32)
            nc.vector.tensor_scalar(qth, uth, -511.0, 0.5,
                                    Alu.is_ge, Alu.mult)
            if not last:
                mth = tinyp.tile([128, KC], f32)
                nc.vector.tensor_scalar(mth, uth, -511.0, 0.5,
                                        Alu.is_lt, Alu.mult)
                nc.vector.scalar_tensor_tensor(vth, uth, 512.0, mth,
                                               Alu.add, Alu.mult)
            # msk = spike * qth01 = g3*(qth01/2) + qth01/2, qth in {0, 0.5}
            msk = mskp.tile([128, KC, HW], bf16)
            for mv in range(KC):
                nc.vector.tensor_scalar(msk[:, mv], sv2s[mv],
                                        qth[:, mv:mv + 1],
                                        qth[:, mv:mv + 1],
                                        Alu.mult, Alu.add)
            return msk

        def tail_stage(b, t, msk):
            last = (t == T - 1)
            pad2 = pad2s[t % npad]
            for mc in range(KC):
                padi = pad2[:, mc].rearrange(
                    "p (h w) -> p h w", h=HP)[:, 1:33, 1:33]
                for nh in range(2):
                    pc = psp.tile([128, 512], f32, tag="ps")
                    mm_half(pc, r2w1T[:, :, mc * 128:(mc + 1) * 128], msk, nh)
                    nc.scalar.activation(
                        padi[:, nh * 16:(nh + 1) * 16, :],
                        pc.rearrange("p (h w) -> p h w", h=16), Act.Copy)
            dwo2 = dwo2p.tile([128, KC, HW], bf16, tag="dwo2")
            for mc in range(KC):
                if (1, mc) in dw_dve:
                    dw_block_dve(dwo2[:, mc], 1, pad2, mc)
                    continue
                for nh in range(2):
                    pd = psp.tile([128, 512], f32, tag="ps")
                    dw_half(pd, diag2, pad2, mc, nh)
                    nc.scalar.activation(
                        dwo2[:, mc, nh * 512:(nh + 1) * 512], pd, Act.Copy)
            for mc in range(KC):
                for nh in range(2):
                    hsl = slice(nh * 512, (nh + 1) * 512)
                    pr = psp.tile([128, 512], f32, tag="ps")
                    mm_half(pr, r2pwT[:, :, mc * 128:(mc + 1) * 128], dwo2, nh)
                    um = ump.tile([128, 512], f32, tag="um")
                    if t == 0:
                        nc.vector.tensor_scalar(
                            um, pr, col(B40, mc), None, Alu.add)
                    else:
                        nc.vector.scalar_tensor_tensor(
                            um, W4[:, mc, hsl], 0.5, pr, Alu.mult, Alu.add)
                    g4 = gp.tile([128, 512], bf16, tag="g")
                    nc.scalar.activation(g4, um, Act.Sign,
                                         bias=cols[:, NEG2:NEG2 + 1])
                    if not last:
                        nc.vector.scalar_tensor_tensor(
                            W4[:, mc, hsl], um, col(C41, mc), g4,
                            Alu.subtract, Alu.subtract)
                    ot = outp.tile([128, 512], f32, tag="ot")
                    nc.vector.tensor_scalar(ot, g4, sc / 2, sc / 2,
                                            Alu.mult, Alu.add)
                    nc.sync.dma_start(
                        out=out_d[t, b].rearrange(
                            "(kc kp) f -> kp kc f", kp=128)[:, mc, hsl],
                        in_=ot)

        import contextlib
        loop_cm = (tc.For_i(0, loop_repeat, 1) if loop_repeat
                   else contextlib.nullcontext())
        with loop_cm:
          for rep in range(repeat):
            pairs = [(b, t) for b in range(BL) for t in range(T)]
            # prologue: lif1/conv1/dw1 for the first (b, t)
            s1 = lif1_stage(*pairs[0])
            conv1_stage(*pairs[0], s1)
            dwo1 = dw1_stage(*pairs[0])
            for i, (b, t) in enumerate(pairs):
                nxt = pairs[i + 1] if i + 1 < len(pairs) else None
                gsum, sv2s = pw1_lif_stage(b, t, dwo1)
                if nxt:
                    s1 = lif1_stage(*nxt)
                    conv1_stage(*nxt, s1)
                msk = th_mask_stage(b, t, gsum, sv2s)
                if nxt:
                    dwo1 = dw1_stage(*nxt)
                tail_stage(b, t, msk)
    nc.finalize()
    return nc


_BUILD_CACHE = {}


def get_nc(sc, repeat=1, **kw):
    key = (float(sc), repeat, tuple(sorted(kw.items())))
    if key not in _BUILD_CACHE:
        _BUILD_CACHE[key] = build(float(sc), repeat, **kw)
    return _BUILD_CACHE[key]


def make_in_maps(inputs):
    x = np.asarray(inputs["x"], np.float32)
    prep = host_prep(**{k: inputs[k] for k in
                        ("r1_w1", "r1_bn1", "r1_dw", "r1_pw", "r1_bn2",
                         "qkv_bn", "r2_w1", "r2_bn1", "r2_dw", "r2_pw",
                         "r2_bn2", "proj_bn")})
    in_maps = []
    for i in range(NCORES):
        shard = np.ascontiguousarray(
            x[:, i * BL:(i + 1) * BL].reshape(T, BL, C, HW))
        in_maps.append({"xs": shard, **prep})
    return in_maps


def kernel(**inputs):
    sc = float(np.asarray(inputs["scale"]).reshape(-1)[0])
    nc = get_nc(sc, pad_db=True)
    in_maps = make_in_maps(inputs)
    res = run_bass_kernel_spmd(nc, in_maps, core_ids=list(range(NCORES)))
    out = np.concatenate([res.results[i]["out"] for i in range(NCORES)],
                         axis=1)
    return out.reshape(T, B, C, H, W)
